# revision 61
# baseline (speedup 1.0000x reference)
"""Trainium2 Bass kernel for nn_ContextualModel_75806172774985.

Per-sample computation (B = 4M samples, S=4 steps, Q=5 features):
    y[b, m] = sum_{s < L[b]} q0[b,s] * (A @ feats[b,s])[m],
    A = W_reg @ W_kernel  (4x4)

Key idea: kernel() re-shards by sequence length. Samples are binned by
L on the host; class 0 (L=0, ~20%) yields y=0 and never touches the
device. Classes 1..4 are packed into fixed 896-column segments (128
samples per column, 7 groups of 128 columns each), with only the first
L steps of x shipped (22.9MB instead of 40.6MB per core). On-device
there is no mask, no seq tensor: m_s = q0_s * feats_s for s < L, the
PE accumulates exactly L terms per group, applies the 4x4 A via the
transpose + block-diagonal trick, and writes bf16 output in transposed
layout (the host inverts the permutation).

Engine split per class-L tile:
    VectorE/GpSimd: L x tensor_mul m_s[k,f] = q0[k,s] * x[k,s,1+f]
                    (bf16 out, k-range split ~1.8 vs ~3.7 ns/el)
    TensorE : per 512-col group: L identity-matmul accumulates -> v,
              4 transposes (bf16), 1 block-diag-A matmul
    ScalarE : 3 PSUM->SBUF copies per group (bf16 casts), output DMA
    Software-pipelined stages: slot s runs accum(s), vcopy(s-1),
    transpose(s-2), vtcopy(s-3), y1mm(s-4), ycopy(s-5) so no engine
    queue ever head-of-line blocks.
"""
import numpy as np
import ml_dtypes

import concourse.bass as bass
import concourse.tile as tile
from concourse import bacc, mybir
from concourse.bass_utils import run_bass_kernel_spmd

N_CORES = 8
P = 128
B_TOTAL = 4_000_000
BS = B_TOTAL // N_CORES          # 500_000 samples per core

f32 = mybir.dt.float32
bf16 = mybir.dt.bfloat16

CLASSES = (1, 2, 3, 4)
# classes 1-3 sized to 768 cols (6 groups); their sample overflow rides in
# class 4 (896 cols) with zeroed extra steps, so no zero-pad columns are
# ever transferred or computed for classes 1-3
CCOLS_BY_CLASS = {1: 768, 2: 768, 3: 768, 4: 896}
CAPS = [CCOLS_BY_CLASS[L] * P for L in CLASSES]   # fill-order capacities
K_TILES_BY_CLASS = {1: (384, 384), 2: (384, 384),
                    3: (384, 384), 4: (384, 384, 128)}
N_GROUPS = sum(CCOLS_BY_CLASS[L] // 128 for L in CLASSES)   # 25 groups
Y_COLS = N_GROUPS * 4 * 128      # per-partition-i output row length
Y_ELEMS = 128 * Y_COLS
X_ELEMS = sum(P * CCOLS_BY_CLASS[L] * 5 * L for L in CLASSES)


def build_nc(num_devices=N_CORES):
    nc = bacc.Bacc("TRN2", target_bir_lowering=False, debug=False,
                   enable_asserts=False, num_devices=num_devices)

    x_d = nc.dram_tensor("xp", [X_ELEMS], bf16, kind="ExternalInput")
    wk_d = nc.dram_tensor("w_kernel", [4, 4], f32, kind="ExternalInput")
    wr_d = nc.dram_tensor("w_reg", [4, 4], f32, kind="ExternalInput")
    y_d = nc.dram_tensor("y", [Y_ELEMS], bf16, kind="ExternalOutput")

    identb_np = np.eye(128, dtype=np.float32).astype(ml_dtypes.bfloat16)
    identb_d = nc.inline_tensor(identb_np, name="ident128b")
    dmask_np = np.kron(np.eye(32, dtype=np.float32), np.ones((4, 4), np.float32))
    dmask_d = nc.inline_tensor(dmask_np, name="blockdiag_mask")

    with tile.TileContext(nc) as tc:
        with (
            tc.tile_pool(name="xin", bufs=8) as xin_pool,
            tc.tile_pool(name="g", bufs=16) as g_pool,
            tc.tile_pool(name="vs", bufs=4) as v_pool,
            tc.tile_pool(name="vts", bufs=4) as vt_pool,
            tc.tile_pool(name="yt", bufs=5) as y_pool,
            tc.tile_pool(name="singles", bufs=1) as singles,
            tc.tile_pool(name="ps_v", bufs=3, space="PSUM") as ps_v,
            tc.tile_pool(name="ps_vt", bufs=3, space="PSUM") as ps_vt,
            tc.tile_pool(name="ps_y1", bufs=2, space="PSUM") as ps_y1,
        ):
            # ---- one-time setup (const DMAs on the ACT queue so the sync
            # queue's first instruction is tile 0's x load) ----
            identb = singles.tile([128, 128], bf16)
            nc.scalar.dma_start(out=identb[:], in_=identb_d.ap())
            dmask = singles.tile([128, 128], f32)
            nc.scalar.dma_start(out=dmask[:], in_=dmask_d.ap())
            wk_s = singles.tile([4, 4], f32)
            nc.scalar.dma_start(out=wk_s[:], in_=wk_d.ap())          # [c, f]
            wr_s = singles.tile([4, 4], f32)
            nc.scalar.dma_start(out=wr_s[:], in_=wr_d.ap().transpose([1, 0]))

            # W_full[4a+f, 4b+m] = sum_c Wk[c,f] * Wreg[m,c] = A[m,f]
            wk_rep = bass.AP(tensor=wk_s.tensor, offset=wk_s.offset,
                             ap=[list(wk_s.ap[0]), [0, 32], [1, 4]])
            wr_rep = bass.AP(tensor=wr_s.tensor, offset=wr_s.offset,
                             ap=[list(wr_s.ap[0]), [0, 32], [1, 4]])
            wkr = singles.tile([4, 128], f32)
            nc.vector.tensor_copy(wkr[:], wk_rep)
            wrr = singles.tile([4, 128], f32)
            nc.vector.tensor_copy(wrr[:], wr_rep)
            wfull_ps = ps_y1.tile([128, 512], f32, tag="y1")
            nc.tensor.matmul(wfull_ps[:, :128], wkr[:], wrr[:])
            w_sb = singles.tile([128, 128], bf16)
            nc.vector.tensor_mul(w_sb[:], wfull_ps[:, :128], dmask[:])

            # ---- job list: tiles interleaved across classes so per-slot
            # engine load is uniform (no heavy class-4 region) ----
            tiles = {}                # (class_idx, tile_idx) -> (L, x_off, K)
            class_base = 0
            gcol0 = {}
            gc = 0
            for ci, L in enumerate(CLASSES):
                kbase = 0
                for ti, K in enumerate(K_TILES_BY_CLASS[L]):
                    tiles[(ci, ti)] = (L, class_base + kbase * 5 * L, K)
                    gcol0[(ci, ti)] = gc
                    gc += K // 128
                    kbase += K
                class_base += P * CCOLS_BY_CLASS[L] * 5 * L
            ORDER = [(ci, ti) for ci in range(4)
                     for ti in range(len(K_TILES_BY_CLASS[CLASSES[ci]]))]
            jobs = []
            tile_meta = []            # (first_job_idx, L, x_off, K)
            for key in ORDER:
                L, xo, K = tiles[key]
                tile_meta.append((len(jobs), L, xo, K))
                for g in range(K // 128):
                    jobs.append({"L": L, "g": g, "gcol": gcol0[key] + g,
                                 "last": g == K // 128 - 1})
            n_jobs = len(jobs)
            first_to_meta = {m[0]: m for m in tile_meta}

            xd0 = x_d.ap()
            yd0 = y_d.ap()
            pending_dma = []          # (due_slot, y_ap, ytile)
            for s in range(n_jobs + 12):
                # --- due output DMAs (sync queue; deps 2+ slots old) ---
                for due, y_ap, yt in [p for p in pending_dma if p[0] <= s]:
                    nc.sync.dma_start(out=y_ap, in_=yt[:])
                pending_dma = [p for p in pending_dma if p[0] > s]
                # --- tile-level ops when a tile's first group arrives ---
                if s < n_jobs and s in first_to_meta:
                    _, L, xo, K = first_to_meta[s]
                    xt = xin_pool.tile([P, K * 5 * L], bf16, tag="x")
                    x_ap = bass.AP(tensor=xd0.tensor, offset=xd0.offset + xo,
                                   ap=[[CCOLS_BY_CLASS[L] * 5 * L, 128],
                                       [1, K * 5 * L]])
                    nc.sync.dma_start(out=xt[:], in_=x_ap)
                    x4 = xt.rearrange("p (k s e) -> p k s e", s=L, e=5)

                    # m_s[k, f] = q0[k, s] * x[k, s, 1+f]  (bf16 out;
                    # k-split DVE/GpSimd by measured rates; DVE also
                    # carries vtcopies in classes 1-2, so smaller share)
                    KA = K * (13 if L <= 2 else 15) // 24
                    ms = []
                    for ss in range(L):
                        m = g_pool.tile([P, K, 4], bf16, tag="m")
                        q0b = x4[:, :, ss, 0:1].broadcast_to([P, K, 4])
                        nc.vector.tensor_mul(m[:, :KA, :], q0b[:, :KA, :],
                                             x4[:, :KA, ss, 1:5])
                        nc.gpsimd.tensor_mul(m[:, KA:, :], q0b[:, KA:, :],
                                             x4[:, KA:, ss, 1:5])
                        ms.append(m.rearrange("p k f -> p (k f)"))
                    ytile = y_pool.tile([P, K * 4], bf16, tag="y")
                    for j in jobs[s:s + K // 128]:
                        j["ms"] = ms
                        j["ytile"] = ytile

                # --- stage accum(s): v = sum_{s<L} M_s ---
                if s < n_jobs:
                    jb = jobs[s]
                    L = jb["L"]
                    sl = slice(jb["g"] * 512, (jb["g"] + 1) * 512)
                    v_ps = ps_v.tile([128, 512], f32, tag="v")
                    for ss in range(L):
                        nc.tensor.matmul(v_ps[:], identb[:], jb["ms"][ss][:, sl],
                                         start=(ss == 0), stop=(ss == L - 1))
                    jb["v_ps"] = v_ps
                # --- stage vcopy(s-1) ---
                if 0 <= s - 1 < n_jobs:
                    jb = jobs[s - 1]
                    v_sb = v_pool.tile([128, 512], bf16, tag="v")
                    nc.scalar.copy(v_sb[:], jb.pop("v_ps")[:])
                    jb["v_sb"] = v_sb
                # --- stage transpose(s-2) ---
                if 0 <= s - 2 < n_jobs:
                    jb = jobs[s - 2]
                    vt_ps = ps_vt.tile([128, 512], bf16, tag="vt")
                    v_sb = jb.pop("v_sb")
                    for j in range(4):
                        cj = slice(j * 128, (j + 1) * 128)
                        nc.tensor.transpose(vt_ps[:, cj], v_sb[:, cj], identb[:])
                    jb["vt_ps"] = vt_ps
                # --- stage vtcopy(s-3) ---
                if 0 <= s - 3 < n_jobs:
                    jb = jobs[s - 3]
                    vt_sb = vt_pool.tile([128, 512], bf16, tag="vt")
                    if jb["L"] <= 2:
                        nc.vector.tensor_copy(vt_sb[:], jb.pop("vt_ps")[:])
                    else:
                        nc.scalar.copy(vt_sb[:], jb.pop("vt_ps")[:])
                    jb["vt_sb"] = vt_sb
                # --- stage y1mm(s-4) ---
                if 0 <= s - 4 < n_jobs:
                    jb = jobs[s - 4]
                    y1_ps = ps_y1.tile([128, 512], f32, tag="y1")
                    nc.tensor.matmul(y1_ps[:], w_sb[:], jb.pop("vt_sb")[:])
                    jb["y1_ps"] = y1_ps
                # --- stage ycopy(s-5) + per-tile output DMA ---
                if 0 <= s - 5 < n_jobs:
                    jb = jobs[s - 5]
                    sl = slice(jb["g"] * 512, (jb["g"] + 1) * 512)
                    nc.scalar.copy(jb["ytile"][:, sl], jb.pop("y1_ps")[:])
                    if jb["last"]:
                        ng = jb["g"] + 1
                        col0 = (jb["gcol"] - jb["g"]) * 512
                        y_ap = bass.AP(tensor=yd0.tensor,
                                       offset=yd0.offset + col0,
                                       ap=[[Y_COLS, 128], [1, ng * 512]])
                        pending_dma.append((s + 5, y_ap, jb["ytile"]))
    nc.compile()
    return nc


_NC_CACHE = None


def _get_nc():
    global _NC_CACHE
    if _NC_CACHE is None:
        _NC_CACHE = build_nc()
    return _NC_CACHE


def _pack_inputs(xss, seq_lengths, W_kernel, W_reg):
    """Bin samples by L, shard classes across cores, pack x per class.
    Classes 1-3 are capacity-truncated; their overflow rides in class 4
    with the extra steps zeroed (exactly equivalent numerics)."""
    x2 = np.ascontiguousarray(xss.reshape(B_TOTAL, 4, 5), dtype=np.float32)
    seq = np.asarray(seq_lengths)
    wk = np.ascontiguousarray(W_kernel, dtype=np.float32)
    wr = np.ascontiguousarray(W_reg, dtype=np.float32)
    core_ids = [[] for _ in range(N_CORES)]
    over = [[] for _ in range(N_CORES)]          # (ids, orig_L) per core
    chunks_by_class = []
    for L in CLASSES:
        idx = np.flatnonzero(seq == L)
        chunks_by_class.append(np.array_split(idx, N_CORES))
    for li, L in enumerate(CLASSES[:3]):
        cap = CAPS[li]
        for c in range(N_CORES):
            ids = chunks_by_class[li][c]
            core_ids[c].append(ids[:cap])
            if len(ids) > cap:
                over[c].append((ids[cap:], L))
    for c in range(N_CORES):
        ids4 = [chunks_by_class[3][c]] + [o[0] for o in over[c]]
        core_ids[c].append(np.concatenate(ids4))
        assert len(core_ids[c][3]) <= CAPS[3], f"class-4 overflow on core {c}"
    in_maps = []
    packs = [np.zeros(X_ELEMS, dtype=ml_dtypes.bfloat16)
             for _ in range(N_CORES)]
    x_off = 0
    for li, L in enumerate(CLASSES):
        C = CCOLS_BY_CLASS[L]
        seg = P * C * 5 * L
        for c in range(N_CORES):
            ids = core_ids[c][li]
            n = len(ids)
            buf = np.zeros((C * P, L * 5), dtype=ml_dtypes.bfloat16)
            xs = x2[ids, :L, :]
            if li == 3:
                # zero the unused steps of overflow samples
                pos = len(chunks_by_class[3][c])
                for oids, oL in over[c]:
                    xs[pos:pos + len(oids), oL:, :] = 0.0
                    pos += len(oids)
            buf[:n] = xs.reshape(n, L * 5).astype(ml_dtypes.bfloat16)
            # fill order t*128+p  ->  DRAM layout [p][t][s][e]
            packs_c = buf.reshape(C, P, L * 5).transpose(1, 0, 2)
            packs[c][x_off:x_off + seg] = packs_c.reshape(-1)
        x_off += seg
    for c in range(N_CORES):
        in_maps.append({"xp": packs[c], "w_kernel": wk, "w_reg": wr})
    return in_maps, core_ids


def _unscramble(y_flat):
    """Invert the transposed DRAM layout -> per-core fill-order [N, 4]."""
    a = np.asarray(y_flat).astype(np.float32)
    a5 = a.reshape(32, 4, N_GROUPS, 4, 128)      # (k'', m, G, j, p)
    out = np.transpose(a5, (4, 2, 3, 0, 1))      # (p, G, j, k'', m)
    # sample fill index = t*128 + p, t = G*128 + j*32 + k''
    out = np.ascontiguousarray(out).reshape(128, N_GROUPS * 128, 4)
    return np.transpose(out, (1, 0, 2)).reshape(128 * N_GROUPS * 128, 4)


def run(xss, seq_lengths, W_kernel, W_reg, trace=False, **spmd_kwargs):
    nc = _get_nc()
    in_maps, core_ids = _pack_inputs(xss, seq_lengths, W_kernel, W_reg)
    res = run_bass_kernel_spmd(nc, in_maps, core_ids=list(range(N_CORES)),
                               trace=trace, **spmd_kwargs)
    out = np.zeros((B_TOTAL, 4), dtype=np.float32)   # class 0 stays 0
    for c in range(N_CORES):
        yc = _unscramble(res.results[c]["y"])
        off = 0
        for li, L in enumerate(CLASSES):
            ids = core_ids[c][li]
            out[ids] = yc[off:off + len(ids)]
            off += CAPS[li]
    return out, res


def kernel(xss, seq_lengths, W_kernel, W_reg):
    out, _ = run(xss, seq_lengths, W_kernel, W_reg)
    return out


# revision 62
# speedup vs baseline: 1.0117x; 1.0117x over previous
"""Trainium2 Bass kernel for nn_ContextualModel_75806172774985.

Per-sample computation (B = 4M samples, S=4 steps, Q=5 features):
    y[b, m] = sum_{s < L[b]} q0[b,s] * (A @ feats[b,s])[m],
    A = W_reg @ W_kernel  (4x4)

Key idea: kernel() re-shards by sequence length. Samples are binned by
L on the host; class 0 (L=0, ~20%) yields y=0 and never touches the
device. Classes 1..4 are packed into fixed 896-column segments (128
samples per column, 7 groups of 128 columns each), with only the first
L steps of x shipped (22.9MB instead of 40.6MB per core). On-device
there is no mask, no seq tensor: m_s = q0_s * feats_s for s < L, the
PE accumulates exactly L terms per group, applies the 4x4 A via the
transpose + block-diagonal trick, and writes bf16 output in transposed
layout (the host inverts the permutation).

Engine split per class-L tile:
    VectorE/GpSimd: L x tensor_mul m_s[k,f] = q0[k,s] * x[k,s,1+f]
                    (bf16 out, k-range split ~1.8 vs ~3.7 ns/el)
    TensorE : per 512-col group: L identity-matmul accumulates -> v,
              4 transposes (bf16), 1 block-diag-A matmul
    ScalarE : 3 PSUM->SBUF copies per group (bf16 casts), output DMA
    Software-pipelined stages: slot s runs accum(s), vcopy(s-1),
    transpose(s-2), vtcopy(s-3), y1mm(s-4), ycopy(s-5) so no engine
    queue ever head-of-line blocks.
"""
import numpy as np
import ml_dtypes

import concourse.bass as bass
import concourse.tile as tile
from concourse import bacc, mybir
from concourse.bass_utils import run_bass_kernel_spmd

N_CORES = 8
P = 128
B_TOTAL = 4_000_000
BS = B_TOTAL // N_CORES          # 500_000 samples per core

f32 = mybir.dt.float32
bf16 = mybir.dt.bfloat16

CLASSES = (1, 2, 3, 4)
# classes 1-3 sized to 768 cols (6 groups); their sample overflow rides in
# class 4 (896 cols) with zeroed extra steps, so no zero-pad columns are
# ever transferred or computed for classes 1-3
CCOLS_BY_CLASS = {1: 768, 2: 768, 3: 768, 4: 896}
CAPS = [CCOLS_BY_CLASS[L] * P for L in CLASSES]   # fill-order capacities
K_TILES_BY_CLASS = {1: (384, 384), 2: (384, 384),
                    3: (384, 384), 4: (384, 384, 128)}
N_GROUPS = sum(CCOLS_BY_CLASS[L] // 128 for L in CLASSES)   # 25 groups
Y_COLS = N_GROUPS * 4 * 128      # per-partition-i output row length
Y_ELEMS = 128 * Y_COLS
X_ELEMS = sum(P * CCOLS_BY_CLASS[L] * 5 * L for L in CLASSES)


def build_nc(num_devices=N_CORES):
    nc = bacc.Bacc("TRN2", target_bir_lowering=False, debug=False,
                   enable_asserts=False, num_devices=num_devices)

    x_d = nc.dram_tensor("xp", [X_ELEMS], bf16, kind="ExternalInput")
    wk_d = nc.dram_tensor("w_kernel", [4, 4], f32, kind="ExternalInput")
    wr_d = nc.dram_tensor("w_reg", [4, 4], f32, kind="ExternalInput")
    y_d = nc.dram_tensor("y", [Y_ELEMS], bf16, kind="ExternalOutput")

    identb_np = np.eye(128, dtype=np.float32).astype(ml_dtypes.bfloat16)
    identb_d = nc.inline_tensor(identb_np, name="ident128b")
    dmask_np = np.kron(np.eye(32, dtype=np.float32), np.ones((4, 4), np.float32))
    dmask_d = nc.inline_tensor(dmask_np, name="blockdiag_mask")

    with tile.TileContext(nc) as tc:
        with (
            tc.tile_pool(name="xin", bufs=8) as xin_pool,
            tc.tile_pool(name="g", bufs=16) as g_pool,
            tc.tile_pool(name="vs", bufs=4) as v_pool,
            tc.tile_pool(name="vts", bufs=4) as vt_pool,
            tc.tile_pool(name="yt", bufs=5) as y_pool,
            tc.tile_pool(name="singles", bufs=1) as singles,
            tc.tile_pool(name="ps_v", bufs=3, space="PSUM") as ps_v,
            tc.tile_pool(name="ps_vt", bufs=3, space="PSUM") as ps_vt,
            tc.tile_pool(name="ps_y1", bufs=2, space="PSUM") as ps_y1,
        ):
            # ---- one-time setup (const DMAs on the ACT queue so the sync
            # queue's first instruction is tile 0's x load) ----
            identb = singles.tile([128, 128], bf16)
            nc.scalar.dma_start(out=identb[:], in_=identb_d.ap())
            dmask = singles.tile([128, 128], f32)
            nc.scalar.dma_start(out=dmask[:], in_=dmask_d.ap())
            wk_s = singles.tile([4, 4], f32)
            nc.scalar.dma_start(out=wk_s[:], in_=wk_d.ap())          # [c, f]
            wr_s = singles.tile([4, 4], f32)
            nc.scalar.dma_start(out=wr_s[:], in_=wr_d.ap().transpose([1, 0]))

            # W_full[4a+f, 4b+m] = sum_c Wk[c,f] * Wreg[m,c] = A[m,f]
            wk_rep = bass.AP(tensor=wk_s.tensor, offset=wk_s.offset,
                             ap=[list(wk_s.ap[0]), [0, 32], [1, 4]])
            wr_rep = bass.AP(tensor=wr_s.tensor, offset=wr_s.offset,
                             ap=[list(wr_s.ap[0]), [0, 32], [1, 4]])
            wkr = singles.tile([4, 128], f32)
            nc.vector.tensor_copy(wkr[:], wk_rep)
            wrr = singles.tile([4, 128], f32)
            nc.vector.tensor_copy(wrr[:], wr_rep)
            wfull_ps = ps_y1.tile([128, 512], f32, tag="y1")
            nc.tensor.matmul(wfull_ps[:, :128], wkr[:], wrr[:])
            w_sb = singles.tile([128, 128], bf16)
            nc.vector.tensor_mul(w_sb[:], wfull_ps[:, :128], dmask[:])

            # ---- job list: tiles interleaved across classes so per-slot
            # engine load is uniform (no heavy class-4 region) ----
            tiles = {}                # (class_idx, tile_idx) -> (L, x_off, K)
            class_base = 0
            gcol0 = {}
            gc = 0
            for ci, L in enumerate(CLASSES):
                kbase = 0
                for ti, K in enumerate(K_TILES_BY_CLASS[L]):
                    tiles[(ci, ti)] = (L, class_base + kbase * 5 * L, K)
                    gcol0[(ci, ti)] = gc
                    gc += K // 128
                    kbase += K
                class_base += P * CCOLS_BY_CLASS[L] * 5 * L
            ORDER = [(ci, ti) for ci in range(4)
                     for ti in range(len(K_TILES_BY_CLASS[CLASSES[ci]]))]
            jobs = []
            tile_meta = []            # (first_job_idx, L, x_off, K)
            for key in ORDER:
                L, xo, K = tiles[key]
                tile_meta.append((len(jobs), L, xo, K))
                for g in range(K // 128):
                    jobs.append({"L": L, "g": g, "gcol": gcol0[key] + g,
                                 "last": g == K // 128 - 1})
            n_jobs = len(jobs)
            first_to_meta = {m[0]: m for m in tile_meta}

            xd0 = x_d.ap()
            yd0 = y_d.ap()
            pending_dma = []          # (due_slot, y_ap, ytile)
            for s in range(n_jobs + 12):
                # --- due output DMAs (sync queue; deps 2+ slots old) ---
                for due, y_ap, yt in [p for p in pending_dma if p[0] <= s]:
                    nc.sync.dma_start(out=y_ap, in_=yt[:])
                pending_dma = [p for p in pending_dma if p[0] > s]
                # --- tile-level ops when a tile's first group arrives ---
                if s < n_jobs and s in first_to_meta:
                    _, L, xo, K = first_to_meta[s]
                    xt = xin_pool.tile([P, K * 5 * L], bf16,
                                       tag=f"x{s}", bufs=1)
                    x_ap = bass.AP(tensor=xd0.tensor, offset=xd0.offset + xo,
                                   ap=[[CCOLS_BY_CLASS[L] * 5 * L, 128],
                                       [1, K * 5 * L]])
                    nc.sync.dma_start(out=xt[:], in_=x_ap)
                    x4 = xt.rearrange("p (k s e) -> p k s e", s=L, e=5)

                    # m_s[k, f] = q0[k, s] * x[k, s, 1+f]  (bf16 out;
                    # k-split DVE/GpSimd by measured rates; DVE also
                    # carries vtcopies in classes 1-2, so smaller share)
                    KA = K * (13 if L <= 2 else 15) // 24
                    ms = []
                    for ss in range(L):
                        m = g_pool.tile([P, K, 4], bf16, tag="m")
                        q0b = x4[:, :, ss, 0:1].broadcast_to([P, K, 4])
                        nc.vector.tensor_mul(m[:, :KA, :], q0b[:, :KA, :],
                                             x4[:, :KA, ss, 1:5])
                        nc.gpsimd.tensor_mul(m[:, KA:, :], q0b[:, KA:, :],
                                             x4[:, KA:, ss, 1:5])
                        ms.append(m.rearrange("p k f -> p (k f)"))
                    ytile = y_pool.tile([P, K * 4], bf16, tag="y")
                    for j in jobs[s:s + K // 128]:
                        j["ms"] = ms
                        j["ytile"] = ytile

                # --- stage accum(s): v = sum_{s<L} M_s ---
                if s < n_jobs:
                    jb = jobs[s]
                    L = jb["L"]
                    sl = slice(jb["g"] * 512, (jb["g"] + 1) * 512)
                    v_ps = ps_v.tile([128, 512], f32, tag="v")
                    for ss in range(L):
                        nc.tensor.matmul(v_ps[:], identb[:], jb["ms"][ss][:, sl],
                                         start=(ss == 0), stop=(ss == L - 1))
                    jb["v_ps"] = v_ps
                # --- stage vcopy(s-1) ---
                if 0 <= s - 1 < n_jobs:
                    jb = jobs[s - 1]
                    v_sb = v_pool.tile([128, 512], bf16, tag="v")
                    nc.scalar.copy(v_sb[:], jb.pop("v_ps")[:])
                    jb["v_sb"] = v_sb
                # --- stage transpose(s-2) ---
                if 0 <= s - 2 < n_jobs:
                    jb = jobs[s - 2]
                    vt_ps = ps_vt.tile([128, 512], bf16, tag="vt")
                    v_sb = jb.pop("v_sb")
                    for j in range(4):
                        cj = slice(j * 128, (j + 1) * 128)
                        nc.tensor.transpose(vt_ps[:, cj], v_sb[:, cj], identb[:])
                    jb["vt_ps"] = vt_ps
                # --- stage vtcopy(s-3) ---
                if 0 <= s - 3 < n_jobs:
                    jb = jobs[s - 3]
                    vt_sb = vt_pool.tile([128, 512], bf16, tag="vt")
                    if jb["L"] <= 2:
                        nc.vector.tensor_copy(vt_sb[:], jb.pop("vt_ps")[:])
                    else:
                        nc.scalar.copy(vt_sb[:], jb.pop("vt_ps")[:])
                    jb["vt_sb"] = vt_sb
                # --- stage y1mm(s-4) ---
                if 0 <= s - 4 < n_jobs:
                    jb = jobs[s - 4]
                    y1_ps = ps_y1.tile([128, 512], f32, tag="y1")
                    nc.tensor.matmul(y1_ps[:], w_sb[:], jb.pop("vt_sb")[:])
                    jb["y1_ps"] = y1_ps
                # --- stage ycopy(s-5) + per-tile output DMA ---
                if 0 <= s - 5 < n_jobs:
                    jb = jobs[s - 5]
                    sl = slice(jb["g"] * 512, (jb["g"] + 1) * 512)
                    nc.scalar.copy(jb["ytile"][:, sl], jb.pop("y1_ps")[:])
                    if jb["last"]:
                        ng = jb["g"] + 1
                        col0 = (jb["gcol"] - jb["g"]) * 512
                        y_ap = bass.AP(tensor=yd0.tensor,
                                       offset=yd0.offset + col0,
                                       ap=[[Y_COLS, 128], [1, ng * 512]])
                        pending_dma.append((s + 5, y_ap, jb["ytile"]))
    nc.compile()
    return nc


_NC_CACHE = None


def _get_nc():
    global _NC_CACHE
    if _NC_CACHE is None:
        _NC_CACHE = build_nc()
    return _NC_CACHE


def _pack_inputs(xss, seq_lengths, W_kernel, W_reg):
    """Bin samples by L, shard classes across cores, pack x per class.
    Classes 1-3 are capacity-truncated; their overflow rides in class 4
    with the extra steps zeroed (exactly equivalent numerics)."""
    x2 = np.ascontiguousarray(xss.reshape(B_TOTAL, 4, 5), dtype=np.float32)
    seq = np.asarray(seq_lengths)
    wk = np.ascontiguousarray(W_kernel, dtype=np.float32)
    wr = np.ascontiguousarray(W_reg, dtype=np.float32)
    core_ids = [[] for _ in range(N_CORES)]
    over = [[] for _ in range(N_CORES)]          # (ids, orig_L) per core
    chunks_by_class = []
    for L in CLASSES:
        idx = np.flatnonzero(seq == L)
        chunks_by_class.append(np.array_split(idx, N_CORES))
    for li, L in enumerate(CLASSES[:3]):
        cap = CAPS[li]
        for c in range(N_CORES):
            ids = chunks_by_class[li][c]
            core_ids[c].append(ids[:cap])
            if len(ids) > cap:
                over[c].append((ids[cap:], L))
    for c in range(N_CORES):
        ids4 = [chunks_by_class[3][c]] + [o[0] for o in over[c]]
        core_ids[c].append(np.concatenate(ids4))
        assert len(core_ids[c][3]) <= CAPS[3], f"class-4 overflow on core {c}"
    in_maps = []
    packs = [np.zeros(X_ELEMS, dtype=ml_dtypes.bfloat16)
             for _ in range(N_CORES)]
    x_off = 0
    for li, L in enumerate(CLASSES):
        C = CCOLS_BY_CLASS[L]
        seg = P * C * 5 * L
        for c in range(N_CORES):
            ids = core_ids[c][li]
            n = len(ids)
            buf = np.zeros((C * P, L * 5), dtype=ml_dtypes.bfloat16)
            xs = x2[ids, :L, :]
            if li == 3:
                # zero the unused steps of overflow samples
                pos = len(chunks_by_class[3][c])
                for oids, oL in over[c]:
                    xs[pos:pos + len(oids), oL:, :] = 0.0
                    pos += len(oids)
            buf[:n] = xs.reshape(n, L * 5).astype(ml_dtypes.bfloat16)
            # fill order t*128+p  ->  DRAM layout [p][t][s][e]
            packs_c = buf.reshape(C, P, L * 5).transpose(1, 0, 2)
            packs[c][x_off:x_off + seg] = packs_c.reshape(-1)
        x_off += seg
    for c in range(N_CORES):
        in_maps.append({"xp": packs[c], "w_kernel": wk, "w_reg": wr})
    return in_maps, core_ids


def _unscramble(y_flat):
    """Invert the transposed DRAM layout -> per-core fill-order [N, 4]."""
    a = np.asarray(y_flat).astype(np.float32)
    a5 = a.reshape(32, 4, N_GROUPS, 4, 128)      # (k'', m, G, j, p)
    out = np.transpose(a5, (4, 2, 3, 0, 1))      # (p, G, j, k'', m)
    # sample fill index = t*128 + p, t = G*128 + j*32 + k''
    out = np.ascontiguousarray(out).reshape(128, N_GROUPS * 128, 4)
    return np.transpose(out, (1, 0, 2)).reshape(128 * N_GROUPS * 128, 4)


def run(xss, seq_lengths, W_kernel, W_reg, trace=False, **spmd_kwargs):
    nc = _get_nc()
    in_maps, core_ids = _pack_inputs(xss, seq_lengths, W_kernel, W_reg)
    res = run_bass_kernel_spmd(nc, in_maps, core_ids=list(range(N_CORES)),
                               trace=trace, **spmd_kwargs)
    out = np.zeros((B_TOTAL, 4), dtype=np.float32)   # class 0 stays 0
    for c in range(N_CORES):
        yc = _unscramble(res.results[c]["y"])
        off = 0
        for li, L in enumerate(CLASSES):
            ids = core_ids[c][li]
            out[ids] = yc[off:off + len(ids)]
            off += CAPS[li]
    return out, res


def kernel(xss, seq_lengths, W_kernel, W_reg):
    out, _ = run(xss, seq_lengths, W_kernel, W_reg)
    return out
